# revision 29
# baseline (speedup 1.0000x reference)
"""Trainium2 Bass kernel for the BettingLoss problem.

Strategy (pure data parallel, 8 NeuronCores):
  - Shard the batch dim into 8 contiguous row blocks, one per core.
  - Encode inputs f16 on host: p16 = probs, o16 = odds with the win bit
    in the SIGN (win -> +odds, loss -> -odds). Win/odds tensors collapse
    into one, and all big DVE ops run in 2-byte 2x mode.
  - Per core, rows are laid out trap-major [P=128, T, rows] per tile and
    packed tile-major in DRAM so each tile's DMA is one fully-contiguous
    per-partition chunk.
  - Per tile (rr rows/partition):
        a    = o16 * p16                      (DVE tt 2x; sign = win)
        best = max_t |a|                      (DVE abs_max/max tree, 2x)
        relu = relu(1.1*best - 1), accum      (ScalarE; sum -> RELU)
        bet  = sign(relu), accum              (ScalarE; sum -> NB)
        bestg= best * bet                     (DVE tt 2x)
        eqm  = (a == bestg_bcast)             (DVE tt 2x; matches only the
                                               argmax trap AND only if won,
                                               since losses have a < 0)
        S   += sum(o16 * eqm)                 (GpSimd stt accum -> S_WO)
  - Host combines partials in float64:
        loss         = -(0.019 * RELU) / B    (fallback if NB == 0)
        batch_profit = 0.0209 * S_WO - 0.019 * NB
        num_bets     = NB
"""

import sys

if "/opt/trn_rl_repo" not in sys.path:
    sys.path.insert(0, "/opt/trn_rl_repo")

import numpy as np

B = 4_194_304
T = 6
N_CORES = 8
BC = B // N_CORES          # rows per core
P = 128                    # SBUF partitions
ROWS_PP = BC // P          # rows per partition per core (4096)
R = 512                    # max rows per partition per tile
ROW_TILES = [256, 256] + [512] * 6 + [384, 128]  # ramp in, small tail
NT = len(ROW_TILES)
FLAT = 2 * T * ROWS_PP     # f16 elements per partition in the packed tensor

ALPHA = 1.1
COMMISSION = 0.05
BET_PCT = 0.02
PAYOUT_SCALE = BET_PCT * (1.0 - COMMISSION)          # 0.019
WIN_COEF = ALPHA * BET_PCT * (1.0 - COMMISSION)      # 0.0209

_PROGRAM = None


def _build_program():
    from concourse import bacc, mybir
    from concourse.tile import TileContext

    F16 = mybir.dt.float16
    F32 = mybir.dt.float32
    Alu = mybir.AluOpType
    Act = mybir.ActivationFunctionType

    nc = bacc.Bacc("TRN2", target_bir_lowering=False, debug=False,
                   num_devices=N_CORES)
    po_d = nc.dram_tensor("po", [P, FLAT], F16, kind="ExternalInput").ap()
    acc_d = nc.dram_tensor("acc", [P, 3 * NT], F32, kind="ExternalOutput").ap()

    with TileContext(nc) as tc:
        with tc.tile_pool(name="io", bufs=6) as io_pool, \
             tc.tile_pool(name="ap", bufs=4) as a_pool, \
             tc.tile_pool(name="sp", bufs=4) as s_pool, \
             tc.tile_pool(name="jp", bufs=4) as j_pool, \
             tc.tile_pool(name="accp", bufs=1) as acc_pool:
            acc = acc_pool.tile([P, 3 * NT], F32)
            neg1 = acc_pool.tile([P, 1], F32)
            nc.vector.memset(neg1[:], -1.0)
            zero = acc_pool.tile([P, 1], F32)
            nc.vector.memset(zero[:], 0.0)
            assert sum(ROW_TILES) == ROWS_PP
            off = 0
            for k, rr in enumerate(ROW_TILES):
                fr = 2 * T * rr
                pot = io_pool.tile([P, 2 * T * R], F16, tag="pot",
                                   name=f"pot{k}")[:, :fr]
                (nc.scalar if k in (1, 3) else nc.sync).dma_start(out=pot, in_=po_d[:, off:off + fr])
                off += fr

                po3 = pot.rearrange("p (c t n) -> p c t n", c=2, t=T)
                pt = po3[:, 0]          # [P, T, rr] probs f16
                ot = po3[:, 1]          # [P, T, rr] odds f16, sign=win

                a = a_pool.tile([P, T, R], F16, tag="a", name=f"a{k}")[:, :, :rr]
                eqm = a_pool.tile([P, T, R], F16, tag="eq", name=f"eq{k}")[:, :, :rr]
                s_t = j_pool.tile([P, T, R], F16, tag="s",
                                  name=f"s{k}")[:, :, :rr]
                m3 = s_pool.tile([P, 3, R], F16, tag="m3", name=f"m3{k}")[:, :, :rr]
                r2 = s_pool.tile([P, R], F16, tag="r2", name=f"r2{k}")[:, :rr]
                best = s_pool.tile([P, R], F16, tag="best", name=f"best{k}")[:, :rr]
                relu_t = s_pool.tile([P, R], F16, tag="relu", name=f"relu{k}")[:, :rr]
                bestg = s_pool.tile([P, R], F16, tag="bestg", name=f"bestg{k}")[:, :rr]

                # a = odds * probs  (>= 0: loss rows have BOTH signs flipped)
                nc.vector.tensor_tensor(a, ot, pt, op=Alu.mult)
                # best = max_t a: 3-op DVE tree, all 2x
                nc.vector.tensor_tensor(m3, a[:, 0:T:2, :], a[:, 1:T:2, :],
                                        op=Alu.max)
                nc.vector.tensor_tensor(r2, m3[:, 0, :], m3[:, 1, :], op=Alu.max)
                nc.vector.tensor_tensor(best, r2, m3[:, 2, :], op=Alu.max)
                # loss term on ScalarE (Relu is the ONLY act fn -> one table load)
                nc.scalar.activation(relu_t, best, Act.Relu, bias=neg1[:],
                                     scale=float(np.float32(ALPHA)),
                                     accum_out=acc[:, k:k + 1])
                # bestg = best where bet else 0; accum gives SB (NB on host)
                nc.vector.scalar_tensor_tensor(bestg, best,
                                               float(np.float32(1.0 / ALPHA)),
                                               best, op0=Alu.is_gt, op1=Alu.mult,
                                               accum_out=acc[:, NT + k:NT + k + 1])
                bestg_b = bestg.unsqueeze(1).broadcast_to([P, T, rr])
                nc.vector.tensor_tensor(eqm, a, bestg_b, op=Alu.is_equal)
                # s = odds(+/-) at the argmax trap; relu-accum keeps win rows
                nc.vector.tensor_tensor(s_t, eqm, ot, op=Alu.mult)
                nc.scalar.activation(s_t, s_t, Act.Relu, bias=zero[:],
                                     accum_out=acc[:, 2 * NT + k:2 * NT + k + 1])

            nc.sync.dma_start(out=acc_d, in_=acc[:])

    nc.compile()
    return nc


def _get_program():
    global _PROGRAM
    if _PROGRAM is None:
        _PROGRAM = _build_program()
    return _PROGRAM


def _pack_core(probs, win, odds, i):
    """Core i's packed [P, FLAT] f16 tensor, tile-major per partition."""
    loss_m = win[i * BC:(i + 1) * BC] <= 0.5
    p16 = probs[i * BC:(i + 1) * BC].astype(np.float16)
    p_u = p16.view(np.uint16).copy()
    p_u[loss_m] |= 0x8000                            # loss -> both negative
    p16 = p_u.view(np.float16).reshape(P, ROWS_PP, T)
    o16 = odds[i * BC:(i + 1) * BC].astype(np.float16)
    o_u = o16.view(np.uint16).copy()
    o_u[loss_m] |= 0x8000
    o16 = o_u.view(np.float16).reshape(P, ROWS_PP, T)

    blocks = []
    r0 = 0
    for rr in ROW_TILES:
        sl = slice(r0, r0 + rr)
        r0 += rr
        # [P, 2, T, rr] for this tile
        blk = np.empty((P, 2, T, rr), np.float16)
        blk[:, 0] = p16[:, sl, :].transpose(0, 2, 1)
        blk[:, 1] = o16[:, sl, :].transpose(0, 2, 1)
        blocks.append(blk.reshape(P, -1))
    return np.ascontiguousarray(np.concatenate(blocks, axis=1))


def _install_ntff_shim():
    """Provide antenv.axon_hooks (missing in this image) so trace=True works."""
    import contextlib
    import ctypes
    import types

    if "antenv.axon_hooks" in sys.modules:
        return
    try:
        from antenv import axon_hooks  # noqa: F401
        return
    except ImportError:
        pass

    so_path = "/opt/axon/libaxon_pjrt.so"
    hook = None
    try:
        lib = ctypes.CDLL(so_path)
        if hasattr(lib, "axon_start_nrt_profile"):
            lib.axon_start_nrt_profile.argtypes = [
                ctypes.POINTER(ctypes.c_int64), ctypes.c_size_t]
            lib.axon_start_nrt_profile.restype = ctypes.c_int64
            lib.axon_stop_nrt_profile.argtypes = [ctypes.c_char_p]
            lib.axon_stop_nrt_profile.restype = ctypes.c_int64

            @contextlib.contextmanager
            def _hook(output_dir, device_ids):
                import jax
                jax.devices()
                if device_ids:
                    ids = (ctypes.c_int64 * len(device_ids))(*device_ids)
                    rc = lib.axon_start_nrt_profile(ids, len(device_ids))
                else:
                    rc = lib.axon_start_nrt_profile(None, 0)
                if rc != 0:
                    raise RuntimeError(f"axon_start_nrt_profile rc={rc}")
                try:
                    yield
                finally:
                    n = lib.axon_stop_nrt_profile(str(output_dir).encode())
                    print(f"profile: {n} file(s) written to {output_dir}",
                          file=sys.stderr)

            hook = _hook
    except OSError:
        pass

    mod = types.ModuleType("antenv.axon_hooks")
    mod.get_axon_ntff_profile_hook = lambda: hook
    mod.set_axon_ntff_profile_hook = lambda h: None
    sys.modules["antenv.axon_hooks"] = mod


def _run_device(predicted_probs, true_winners, market_odds, trace=False):
    from concourse.bass_utils import run_bass_kernel_spmd

    if trace:
        _install_ntff_shim()
    nc = _get_program()
    in_maps = []
    for i in range(N_CORES):
        in_maps.append({
            "po": _pack_core(predicted_probs, true_winners, market_odds, i),
        })
    res = run_bass_kernel_spmd(nc, in_maps, list(range(N_CORES)), trace=trace)
    return res


def kernel(predicted_probs, true_winners, market_odds, _trace=False,
           _result_holder=None):
    res = _run_device(predicted_probs, true_winners, market_odds, trace=_trace)
    if _result_holder is not None:
        _result_holder.append(res)

    RELU = 0.0
    SB = 0.0
    S_WO = 0.0
    for i in range(N_CORES):
        a_s = res.results[i]["acc"].astype(np.float64)
        RELU += a_s[:, :NT].sum()
        SB += a_s[:, NT:2 * NT].sum()
        S_WO += a_s[:, 2 * NT:].sum()
    num_bets = max(0, int(round(ALPHA * SB - RELU)))

    if num_bets > 0:
        total_expected_profit = PAYOUT_SCALE * RELU
    else:
        total_expected_profit = -np.float64(
            np.mean(np.max(predicted_probs, axis=1))) * 0.1
    loss = -total_expected_profit / B
    batch_profit = WIN_COEF * S_WO - PAYOUT_SCALE * num_bets

    return (np.float32(loss), np.float32(batch_profit), np.int32(num_bets))


if __name__ == "__main__":
    rng = np.random.default_rng(0)
    probs = rng.random((B, T), dtype=np.float32)
    win = (rng.random((B, T)) > 0.8).astype(np.float32)
    odds = rng.random((B, T), dtype=np.float32) * 10.0
    odds[rng.random((B, 1))[:, 0] < 0.1] = 0.0
    out = kernel(probs, win, odds)
    print("kernel out:", out)


# revision 30
# speedup vs baseline: 1.2156x; 1.2156x over previous
"""Trainium2 Bass kernel for the BettingLoss problem.

Strategy (pure data parallel, 8 NeuronCores):
  - Shard the batch dim into 8 contiguous row blocks, one per core.
  - Encode inputs f16 on host: p16 = probs, o16 = odds with the win bit
    in the SIGN (win -> +odds, loss -> -odds). Win/odds tensors collapse
    into one, and all big DVE ops run in 2-byte 2x mode.
  - Per core, rows are laid out trap-major [P=128, T, rows] per tile and
    packed tile-major in DRAM so each tile's DMA is one fully-contiguous
    per-partition chunk.
  - Per tile (rr rows/partition):
        a    = o16 * p16                      (DVE tt 2x; sign = win)
        best = max_t |a|                      (DVE abs_max/max tree, 2x)
        relu = relu(1.1*best - 1), accum      (ScalarE; sum -> RELU)
        bet  = sign(relu), accum              (ScalarE; sum -> NB)
        bestg= best * bet                     (DVE tt 2x)
        eqm  = (a == bestg_bcast)             (DVE tt 2x; matches only the
                                               argmax trap AND only if won,
                                               since losses have a < 0)
        S   += sum(o16 * eqm)                 (GpSimd stt accum -> S_WO)
  - Host combines partials in float64:
        loss         = -(0.019 * RELU) / B    (fallback if NB == 0)
        batch_profit = 0.0209 * S_WO - 0.019 * NB
        num_bets     = NB
"""

import sys

if "/opt/trn_rl_repo" not in sys.path:
    sys.path.insert(0, "/opt/trn_rl_repo")

import numpy as np

B = 4_194_304
T = 6
N_CORES = 8
BC = B // N_CORES          # rows per core
P = 128                    # SBUF partitions
ROWS_PP = BC // P          # rows per partition per core (4096)
R = 512                    # max rows per partition per tile
ROW_TILES = [256, 256] + [512] * 6 + [384, 128]  # ramp in, small tail
NT = len(ROW_TILES)
FLAT = 2 * T * ROWS_PP     # f16 elements per partition in the packed tensor

ALPHA = 1.1
COMMISSION = 0.05
BET_PCT = 0.02
PAYOUT_SCALE = BET_PCT * (1.0 - COMMISSION)          # 0.019
WIN_COEF = ALPHA * BET_PCT * (1.0 - COMMISSION)      # 0.0209

_PROGRAM = None


def _build_program():
    from concourse import bacc, mybir
    from concourse.tile import TileContext

    F16 = mybir.dt.float16
    F32 = mybir.dt.float32
    Alu = mybir.AluOpType
    Act = mybir.ActivationFunctionType

    nc = bacc.Bacc("TRN2", target_bir_lowering=False, debug=False,
                   num_devices=N_CORES)
    po_d = nc.dram_tensor("po", [P, FLAT], F16, kind="ExternalInput").ap()
    acc_d = nc.dram_tensor("acc", [P, 3 * NT], F32, kind="ExternalOutput").ap()

    with TileContext(nc) as tc:
        with tc.tile_pool(name="io", bufs=6) as io_pool, \
             tc.tile_pool(name="ap", bufs=4) as a_pool, \
             tc.tile_pool(name="sp", bufs=4) as s_pool, \
             tc.tile_pool(name="jp", bufs=4) as j_pool, \
             tc.tile_pool(name="accp", bufs=1) as acc_pool:
            acc = acc_pool.tile([P, 3 * NT], F32)
            neg1 = acc_pool.tile([P, 1], F32)
            nc.vector.memset(neg1[:], -1.0)
            zero = acc_pool.tile([P, 1], F32)
            nc.vector.memset(zero[:], 0.0)
            assert sum(ROW_TILES) == ROWS_PP
            off = 0
            for k, rr in enumerate(ROW_TILES):
                fr = 2 * T * rr
                pot = io_pool.tile([P, 2 * T * R], F16, tag="pot",
                                   name=f"pot{k}")[:, :fr]
                nc.sync.dma_start(out=pot, in_=po_d[:, off:off + fr])
                off += fr

                po3 = pot.rearrange("p (c t n) -> p c t n", c=2, t=T)
                pt = po3[:, 0]          # [P, T, rr] probs f16
                ot = po3[:, 1]          # [P, T, rr] odds f16, sign=win

                a = a_pool.tile([P, T, R], F16, tag="a", name=f"a{k}")[:, :, :rr]
                eqm = a_pool.tile([P, T, R], F16, tag="eq", name=f"eq{k}")[:, :, :rr]
                s_t = j_pool.tile([P, T, R], F16, tag="s",
                                  name=f"s{k}")[:, :, :rr]
                m3 = s_pool.tile([P, 3, R], F16, tag="m3", name=f"m3{k}")[:, :, :rr]
                r2 = s_pool.tile([P, R], F16, tag="r2", name=f"r2{k}")[:, :rr]
                best = s_pool.tile([P, R], F16, tag="best", name=f"best{k}")[:, :rr]
                relu_t = s_pool.tile([P, R], F16, tag="relu", name=f"relu{k}")[:, :rr]
                bestg = s_pool.tile([P, R], F16, tag="bestg", name=f"bestg{k}")[:, :rr]

                # a = odds * probs  (>= 0: loss rows have BOTH signs flipped)
                nc.vector.tensor_tensor(a, ot, pt, op=Alu.mult)
                # best = max_t a: 3-op DVE tree, all 2x
                nc.vector.tensor_tensor(m3, a[:, 0:T:2, :], a[:, 1:T:2, :],
                                        op=Alu.max)
                nc.vector.tensor_tensor(r2, m3[:, 0, :], m3[:, 1, :], op=Alu.max)
                nc.vector.tensor_tensor(best, r2, m3[:, 2, :], op=Alu.max)
                # loss term on ScalarE (Relu is the ONLY act fn -> one table load)
                nc.scalar.activation(relu_t, best, Act.Relu, bias=neg1[:],
                                     scale=float(np.float32(ALPHA)),
                                     accum_out=acc[:, k:k + 1])
                # bestg = best where bet else 0; accum gives SB (NB on host)
                nc.vector.scalar_tensor_tensor(bestg, best,
                                               float(np.float32(1.0 / ALPHA)),
                                               best, op0=Alu.is_gt, op1=Alu.mult,
                                               accum_out=acc[:, NT + k:NT + k + 1])
                bestg_b = bestg.unsqueeze(1).broadcast_to([P, T, rr])
                nc.vector.tensor_tensor(eqm, a, bestg_b, op=Alu.is_equal)
                # s = odds(+/-) at the argmax trap; relu-accum keeps win rows
                nc.vector.tensor_tensor(s_t, eqm, ot, op=Alu.mult)
                nc.scalar.activation(s_t, s_t, Act.Relu, bias=zero[:],
                                     accum_out=acc[:, 2 * NT + k:2 * NT + k + 1])

            nc.sync.dma_start(out=acc_d, in_=acc[:])

    nc.compile()
    return nc


def _get_program():
    global _PROGRAM
    if _PROGRAM is None:
        _PROGRAM = _build_program()
    return _PROGRAM


def _pack_core(probs, win, odds, i):
    """Core i's packed [P, FLAT] f16 tensor, tile-major per partition."""
    loss_m = win[i * BC:(i + 1) * BC] <= 0.5
    p16 = probs[i * BC:(i + 1) * BC].astype(np.float16)
    p_u = p16.view(np.uint16).copy()
    p_u[loss_m] |= 0x8000                            # loss -> both negative
    p16 = p_u.view(np.float16).reshape(P, ROWS_PP, T)
    o16 = odds[i * BC:(i + 1) * BC].astype(np.float16)
    o_u = o16.view(np.uint16).copy()
    o_u[loss_m] |= 0x8000
    o16 = o_u.view(np.float16).reshape(P, ROWS_PP, T)

    blocks = []
    r0 = 0
    for rr in ROW_TILES:
        sl = slice(r0, r0 + rr)
        r0 += rr
        # [P, 2, T, rr] for this tile
        blk = np.empty((P, 2, T, rr), np.float16)
        blk[:, 0] = p16[:, sl, :].transpose(0, 2, 1)
        blk[:, 1] = o16[:, sl, :].transpose(0, 2, 1)
        blocks.append(blk.reshape(P, -1))
    return np.ascontiguousarray(np.concatenate(blocks, axis=1))


def _install_ntff_shim():
    """Provide antenv.axon_hooks (missing in this image) so trace=True works."""
    import contextlib
    import ctypes
    import types

    if "antenv.axon_hooks" in sys.modules:
        return
    try:
        from antenv import axon_hooks  # noqa: F401
        return
    except ImportError:
        pass

    so_path = "/opt/axon/libaxon_pjrt.so"
    hook = None
    try:
        lib = ctypes.CDLL(so_path)
        if hasattr(lib, "axon_start_nrt_profile"):
            lib.axon_start_nrt_profile.argtypes = [
                ctypes.POINTER(ctypes.c_int64), ctypes.c_size_t]
            lib.axon_start_nrt_profile.restype = ctypes.c_int64
            lib.axon_stop_nrt_profile.argtypes = [ctypes.c_char_p]
            lib.axon_stop_nrt_profile.restype = ctypes.c_int64

            @contextlib.contextmanager
            def _hook(output_dir, device_ids):
                import jax
                jax.devices()
                if device_ids:
                    ids = (ctypes.c_int64 * len(device_ids))(*device_ids)
                    rc = lib.axon_start_nrt_profile(ids, len(device_ids))
                else:
                    rc = lib.axon_start_nrt_profile(None, 0)
                if rc != 0:
                    raise RuntimeError(f"axon_start_nrt_profile rc={rc}")
                try:
                    yield
                finally:
                    n = lib.axon_stop_nrt_profile(str(output_dir).encode())
                    print(f"profile: {n} file(s) written to {output_dir}",
                          file=sys.stderr)

            hook = _hook
    except OSError:
        pass

    mod = types.ModuleType("antenv.axon_hooks")
    mod.get_axon_ntff_profile_hook = lambda: hook
    mod.set_axon_ntff_profile_hook = lambda h: None
    sys.modules["antenv.axon_hooks"] = mod


def _run_device(predicted_probs, true_winners, market_odds, trace=False):
    from concourse.bass_utils import run_bass_kernel_spmd

    if trace:
        _install_ntff_shim()
    nc = _get_program()
    in_maps = []
    for i in range(N_CORES):
        in_maps.append({
            "po": _pack_core(predicted_probs, true_winners, market_odds, i),
        })
    res = run_bass_kernel_spmd(nc, in_maps, list(range(N_CORES)), trace=trace)
    return res


def kernel(predicted_probs, true_winners, market_odds, _trace=False,
           _result_holder=None):
    res = _run_device(predicted_probs, true_winners, market_odds, trace=_trace)
    if _result_holder is not None:
        _result_holder.append(res)

    RELU = 0.0
    SB = 0.0
    S_WO = 0.0
    for i in range(N_CORES):
        a_s = res.results[i]["acc"].astype(np.float64)
        RELU += a_s[:, :NT].sum()
        SB += a_s[:, NT:2 * NT].sum()
        S_WO += a_s[:, 2 * NT:].sum()
    num_bets = max(0, int(round(ALPHA * SB - RELU)))

    if num_bets > 0:
        total_expected_profit = PAYOUT_SCALE * RELU
    else:
        total_expected_profit = -np.float64(
            np.mean(np.max(predicted_probs, axis=1))) * 0.1
    loss = -total_expected_profit / B
    batch_profit = WIN_COEF * S_WO - PAYOUT_SCALE * num_bets

    return (np.float32(loss), np.float32(batch_profit), np.int32(num_bets))


if __name__ == "__main__":
    rng = np.random.default_rng(0)
    probs = rng.random((B, T), dtype=np.float32)
    win = (rng.random((B, T)) > 0.8).astype(np.float32)
    odds = rng.random((B, T), dtype=np.float32) * 10.0
    odds[rng.random((B, 1))[:, 0] < 0.1] = 0.0
    out = kernel(probs, win, odds)
    print("kernel out:", out)


# revision 31
# speedup vs baseline: 1.2163x; 1.0006x over previous
"""Trainium2 Bass kernel for the BettingLoss problem.

Strategy (pure data parallel, 8 NeuronCores):
  - Shard the batch dim into 8 contiguous row blocks, one per core.
  - Encode inputs f16 on host: p16 = probs, o16 = odds with the win bit
    in the SIGN (win -> +odds, loss -> -odds). Win/odds tensors collapse
    into one, and all big DVE ops run in 2-byte 2x mode.
  - Per core, rows are laid out trap-major [P=128, T, rows] per tile and
    packed tile-major in DRAM so each tile's DMA is one fully-contiguous
    per-partition chunk.
  - Per tile (rr rows/partition):
        a    = o16 * p16                      (DVE tt 2x; sign = win)
        best = max_t |a|                      (DVE abs_max/max tree, 2x)
        relu = relu(1.1*best - 1), accum      (ScalarE; sum -> RELU)
        bet  = sign(relu), accum              (ScalarE; sum -> NB)
        bestg= best * bet                     (DVE tt 2x)
        eqm  = (a == bestg_bcast)             (DVE tt 2x; matches only the
                                               argmax trap AND only if won,
                                               since losses have a < 0)
        S   += sum(o16 * eqm)                 (GpSimd stt accum -> S_WO)
  - Host combines partials in float64:
        loss         = -(0.019 * RELU) / B    (fallback if NB == 0)
        batch_profit = 0.0209 * S_WO - 0.019 * NB
        num_bets     = NB
"""

import sys

if "/opt/trn_rl_repo" not in sys.path:
    sys.path.insert(0, "/opt/trn_rl_repo")

import numpy as np

B = 4_194_304
T = 6
N_CORES = 8
BC = B // N_CORES          # rows per core
P = 128                    # SBUF partitions
ROWS_PP = BC // P          # rows per partition per core (4096)
R = 512                    # max rows per partition per tile
ROW_TILES = [256, 256] + [512] * 6 + [384, 128]  # ramp in, small tail
NT = len(ROW_TILES)
FLAT = 2 * T * ROWS_PP     # f16 elements per partition in the packed tensor

ALPHA = 1.1
COMMISSION = 0.05
BET_PCT = 0.02
PAYOUT_SCALE = BET_PCT * (1.0 - COMMISSION)          # 0.019
WIN_COEF = ALPHA * BET_PCT * (1.0 - COMMISSION)      # 0.0209

_PROGRAM = None


def _build_program():
    from concourse import bacc, mybir
    from concourse.tile import TileContext

    F16 = mybir.dt.float16
    F32 = mybir.dt.float32
    Alu = mybir.AluOpType
    Act = mybir.ActivationFunctionType

    nc = bacc.Bacc("TRN2", target_bir_lowering=False, debug=False,
                   num_devices=N_CORES)
    po_d = nc.dram_tensor("po", [P, FLAT], F16, kind="ExternalInput").ap()
    acc_d = nc.dram_tensor("acc", [P, 3 * NT], F32, kind="ExternalOutput").ap()

    with TileContext(nc) as tc:
        with tc.tile_pool(name="io", bufs=5) as io_pool, \
             tc.tile_pool(name="ap", bufs=4) as a_pool, \
             tc.tile_pool(name="sp", bufs=6) as s_pool, \
             tc.tile_pool(name="jp", bufs=6) as j_pool, \
             tc.tile_pool(name="accp", bufs=1) as acc_pool:
            acc = acc_pool.tile([P, 3 * NT], F32)
            neg1 = acc_pool.tile([P, 1], F32)
            nc.vector.memset(neg1[:], -1.0)
            zero = acc_pool.tile([P, 1], F32)
            nc.vector.memset(zero[:], 0.0)
            assert sum(ROW_TILES) == ROWS_PP
            off = 0
            for k, rr in enumerate(ROW_TILES):
                fr = 2 * T * rr
                pot = io_pool.tile([P, 2 * T * R], F16, tag="pot",
                                   name=f"pot{k}")[:, :fr]
                nc.sync.dma_start(out=pot, in_=po_d[:, off:off + fr])
                off += fr

                po3 = pot.rearrange("p (c t n) -> p c t n", c=2, t=T)
                pt = po3[:, 0]          # [P, T, rr] probs f16
                ot = po3[:, 1]          # [P, T, rr] odds f16, sign=win

                a = a_pool.tile([P, T, R], F16, tag="a", name=f"a{k}")[:, :, :rr]
                eqm = a_pool.tile([P, T, R], F16, tag="eq", name=f"eq{k}")[:, :, :rr]
                s_t = j_pool.tile([P, T, R], F16, tag="s",
                                  name=f"s{k}")[:, :, :rr]
                m3 = s_pool.tile([P, 3, R], F16, tag="m3", name=f"m3{k}")[:, :, :rr]
                r2 = s_pool.tile([P, R], F16, tag="r2", name=f"r2{k}")[:, :rr]
                best = s_pool.tile([P, R], F16, tag="best", name=f"best{k}")[:, :rr]
                relu_t = s_pool.tile([P, R], F16, tag="relu", name=f"relu{k}")[:, :rr]
                bestg = s_pool.tile([P, R], F16, tag="bestg", name=f"bestg{k}")[:, :rr]

                # a = odds * probs  (>= 0: loss rows have BOTH signs flipped)
                nc.vector.tensor_tensor(a, ot, pt, op=Alu.mult)
                # best = max_t a: 3-op DVE tree, all 2x
                nc.vector.tensor_tensor(m3, a[:, 0:T:2, :], a[:, 1:T:2, :],
                                        op=Alu.max)
                nc.vector.tensor_tensor(r2, m3[:, 0, :], m3[:, 1, :], op=Alu.max)
                nc.vector.tensor_tensor(best, r2, m3[:, 2, :], op=Alu.max)
                # loss term on ScalarE (Relu is the ONLY act fn -> one table load)
                nc.scalar.activation(relu_t, best, Act.Relu, bias=neg1[:],
                                     scale=float(np.float32(ALPHA)),
                                     accum_out=acc[:, k:k + 1])
                # bestg = best where bet else 0; accum gives SB (NB on host)
                nc.vector.scalar_tensor_tensor(bestg, best,
                                               float(np.float32(1.0 / ALPHA)),
                                               best, op0=Alu.is_gt, op1=Alu.mult,
                                               accum_out=acc[:, NT + k:NT + k + 1])
                bestg_b = bestg.unsqueeze(1).broadcast_to([P, T, rr])
                nc.vector.tensor_tensor(eqm, a, bestg_b, op=Alu.is_equal)
                # s = odds(+/-) at the argmax trap; relu-accum keeps win rows
                nc.vector.tensor_tensor(s_t, eqm, ot, op=Alu.mult)
                nc.scalar.activation(s_t, s_t, Act.Relu, bias=zero[:],
                                     accum_out=acc[:, 2 * NT + k:2 * NT + k + 1])

            nc.sync.dma_start(out=acc_d, in_=acc[:])

    nc.compile()
    return nc


def _get_program():
    global _PROGRAM
    if _PROGRAM is None:
        _PROGRAM = _build_program()
    return _PROGRAM


def _pack_core(probs, win, odds, i):
    """Core i's packed [P, FLAT] f16 tensor, tile-major per partition."""
    loss_m = win[i * BC:(i + 1) * BC] <= 0.5
    p16 = probs[i * BC:(i + 1) * BC].astype(np.float16)
    p_u = p16.view(np.uint16).copy()
    p_u[loss_m] |= 0x8000                            # loss -> both negative
    p16 = p_u.view(np.float16).reshape(P, ROWS_PP, T)
    o16 = odds[i * BC:(i + 1) * BC].astype(np.float16)
    o_u = o16.view(np.uint16).copy()
    o_u[loss_m] |= 0x8000
    o16 = o_u.view(np.float16).reshape(P, ROWS_PP, T)

    blocks = []
    r0 = 0
    for rr in ROW_TILES:
        sl = slice(r0, r0 + rr)
        r0 += rr
        # [P, 2, T, rr] for this tile
        blk = np.empty((P, 2, T, rr), np.float16)
        blk[:, 0] = p16[:, sl, :].transpose(0, 2, 1)
        blk[:, 1] = o16[:, sl, :].transpose(0, 2, 1)
        blocks.append(blk.reshape(P, -1))
    return np.ascontiguousarray(np.concatenate(blocks, axis=1))


def _install_ntff_shim():
    """Provide antenv.axon_hooks (missing in this image) so trace=True works."""
    import contextlib
    import ctypes
    import types

    if "antenv.axon_hooks" in sys.modules:
        return
    try:
        from antenv import axon_hooks  # noqa: F401
        return
    except ImportError:
        pass

    so_path = "/opt/axon/libaxon_pjrt.so"
    hook = None
    try:
        lib = ctypes.CDLL(so_path)
        if hasattr(lib, "axon_start_nrt_profile"):
            lib.axon_start_nrt_profile.argtypes = [
                ctypes.POINTER(ctypes.c_int64), ctypes.c_size_t]
            lib.axon_start_nrt_profile.restype = ctypes.c_int64
            lib.axon_stop_nrt_profile.argtypes = [ctypes.c_char_p]
            lib.axon_stop_nrt_profile.restype = ctypes.c_int64

            @contextlib.contextmanager
            def _hook(output_dir, device_ids):
                import jax
                jax.devices()
                if device_ids:
                    ids = (ctypes.c_int64 * len(device_ids))(*device_ids)
                    rc = lib.axon_start_nrt_profile(ids, len(device_ids))
                else:
                    rc = lib.axon_start_nrt_profile(None, 0)
                if rc != 0:
                    raise RuntimeError(f"axon_start_nrt_profile rc={rc}")
                try:
                    yield
                finally:
                    n = lib.axon_stop_nrt_profile(str(output_dir).encode())
                    print(f"profile: {n} file(s) written to {output_dir}",
                          file=sys.stderr)

            hook = _hook
    except OSError:
        pass

    mod = types.ModuleType("antenv.axon_hooks")
    mod.get_axon_ntff_profile_hook = lambda: hook
    mod.set_axon_ntff_profile_hook = lambda h: None
    sys.modules["antenv.axon_hooks"] = mod


def _run_device(predicted_probs, true_winners, market_odds, trace=False):
    from concourse.bass_utils import run_bass_kernel_spmd

    if trace:
        _install_ntff_shim()
    nc = _get_program()
    in_maps = []
    for i in range(N_CORES):
        in_maps.append({
            "po": _pack_core(predicted_probs, true_winners, market_odds, i),
        })
    res = run_bass_kernel_spmd(nc, in_maps, list(range(N_CORES)), trace=trace)
    return res


def kernel(predicted_probs, true_winners, market_odds, _trace=False,
           _result_holder=None):
    res = _run_device(predicted_probs, true_winners, market_odds, trace=_trace)
    if _result_holder is not None:
        _result_holder.append(res)

    RELU = 0.0
    SB = 0.0
    S_WO = 0.0
    for i in range(N_CORES):
        a_s = res.results[i]["acc"].astype(np.float64)
        RELU += a_s[:, :NT].sum()
        SB += a_s[:, NT:2 * NT].sum()
        S_WO += a_s[:, 2 * NT:].sum()
    num_bets = max(0, int(round(ALPHA * SB - RELU)))

    if num_bets > 0:
        total_expected_profit = PAYOUT_SCALE * RELU
    else:
        total_expected_profit = -np.float64(
            np.mean(np.max(predicted_probs, axis=1))) * 0.1
    loss = -total_expected_profit / B
    batch_profit = WIN_COEF * S_WO - PAYOUT_SCALE * num_bets

    return (np.float32(loss), np.float32(batch_profit), np.int32(num_bets))


if __name__ == "__main__":
    rng = np.random.default_rng(0)
    probs = rng.random((B, T), dtype=np.float32)
    win = (rng.random((B, T)) > 0.8).astype(np.float32)
    odds = rng.random((B, T), dtype=np.float32) * 10.0
    odds[rng.random((B, 1))[:, 0] < 0.1] = 0.0
    out = kernel(probs, win, odds)
    print("kernel out:", out)
